# revision 1
# baseline (speedup 1.0000x reference)
"""Trainium2 Bass kernel for a decoupled-MoE 1x1-conv container.

Math (per sample b):
    out[b] = (W_shared + weights[b] * W_routed[idx[b]]) @ x[b]
             + (b_shared + weights[b] * b_routed[idx[b]])

Strategy: data-parallel over batch B=128 across 8 NeuronCores (16 samples
per core). The expert gather/combine is a tiny O(B*C^2) host-side
computation over host-visible routing inputs; each core receives its
per-pair block-diagonal combined-weight bank [128, 8, 128] (2 samples per
128x128 lhsT tile) plus a bias bank, both precomputed on host. Activations
stream through the core in bf16 (tolerance is 2e-2; bf16 end-to-end error
is ~4e-3), which halves the HBM traffic of the memory-bound main loop:
~12.9 MB per core at ~360 GB/s. All x tiles, out tiles, and the weight
bank fit in SBUF simultaneously (~13 MB), so the DMA pipeline has no
buffer-recycle stalls: loads are issued up front and the store queue never
blocks a load.
"""

import numpy as np
import ml_dtypes

import concourse.bass as bass
import concourse.mybir as mybir
import concourse.tile as tile
from concourse.bass_utils import run_bass_kernel_spmd

F32 = mybir.dt.float32
BF16 = mybir.dt.bfloat16
NP_BF16 = ml_dtypes.bfloat16

N_CORES = 8
B = 128
C = 64  # C_IN == C_OUT == 64
H = 56
W_ = 56
HW = H * W_  # 3136
B_LOC = B // N_CORES  # 16 samples per core
PAIRS = B_LOC // 2  # 8 pairs -> [128, HW] tiles
CHUNK = 448  # 7 chunks of 448 = 3136, one PSUM bank each
N_CHUNKS = HW // CHUNK


def _legalize_waits(nc, dma_limit=1):
    """Walrus on this target allows a single sync-wait slot per engine
    compute instruction (sequencer-only instructions like InstDrain take
    many). Split excess waits onto same-engine NOPs inserted just before
    the offending instruction — semantically identical: the engine queue
    blocks on each wait in turn before executing the instruction."""
    import bass_rust

    counter = [0]
    for fn in nc.m.functions:
        for blk in fn.blocks:
            new_insts = []
            for inst in blk.instructions:
                si = inst.sync_info
                tname = type(inst).__name__
                limit = dma_limit if tname == "InstDMACopy" else 1
                if si is not None and si.on_wait and len(si.on_wait) > limit:
                    waits = list(si.on_wait)
                    keep = waits[-limit:]
                    extra = waits[:-limit]
                    for w in extra:
                        nop = mybir.InstNoOp(
                            name=f"lgl-nop-{counter[0]}", ins=[], outs=[]
                        )
                        counter[0] += 1
                        nop.engine = inst.engine
                        nop.sync_info = bass_rust.SyncInfo(
                            on_wait=[w], on_update=[]
                        )
                        new_insts.append(nop)
                    si.on_wait = keep
                new_insts.append(inst)
            blk.instructions = new_insts
    return nc


def build_program(legalize=True, nreps=1):
    nc = bass.Bass(
        "TRN2", target_bir_lowering=False, debug=False, use_seq_codegen=True
    )

    x_d = nc.dram_tensor("x", [PAIRS, 2 * C, HW], BF16, kind="ExternalInput")
    bd_d = nc.dram_tensor("bd", [2 * C, PAIRS, 2 * C], BF16, kind="ExternalInput")
    bias_d = nc.dram_tensor("bias2", [2 * C, PAIRS], F32, kind="ExternalInput")
    out_d = nc.dram_tensor("out", [PAIRS, 2 * C, HW], BF16, kind="ExternalOutput")

    with tile.TileContext(nc) as tc:
        with (
            tc.tile_pool(name="keep", bufs=1) as keep,
            tc.tile_pool(name="xs", bufs=1) as xs,
            tc.tile_pool(name="os", bufs=1) as osp,
            tc.tile_pool(name="pp", bufs=8, space="PSUM") as pp,
        ):
            # Weight/bias banks: tiny; issued from the ACT queue so the SP
            # queue's first issue is already x0 (keeps the big-DMA stream
            # head as early as possible).
            bd = keep.tile([2 * C, PAIRS, 2 * C], BF16)
            nc.scalar.dma_start(bd, bd_d.ap())
            bias2 = keep.tile([2 * C, PAIRS], F32)
            nc.scalar.dma_start(bias2, bias_d.ap())

            # PE p-state warmup: the PE ramps (pstate low/mid) for its first
            # ~3us of activity, which would otherwise slow pair-0's chunks
            # and delay the first store (the write stream's start). Burn the
            # idle head window with dummy full-width matmuls. Sized to 7 so
            # the chain ends (~4.1us) just past the EARLIEST plausible HW
            # arrival of x0+bd (~3.4us with duplex DMA): it bridges into the
            # first real matmul when loads are fast, and merely idles out —
            # costing nothing — when they are slow. A longer chain could
            # overshoot a fast gate and delay pair 0 more than the warm
            # chunks save.
            ws = keep.tile([2 * C, CHUNK], BF16)
            nc.gpsimd.memset(ws, 0.0)
            # Warmup shares the main chunk tag ("ps") so its banks return
            # to the 8-deep rotation; the WAW reuse deps are satisfied long
            # before the main loop reaches them.
            for _ in range(7):
                wps = pp.tile([2 * C, CHUNK], F32, name="ps")
                nc.tensor.matmul(
                    wps[:8, :], ws[:, :8], ws, start=True, stop=True
                )

            # All x tiles persistent in SBUF: issue every load up front on
            # the SP queue; nothing ever waits to reuse these buffers.
            xts = []
            ots = []
            for pr in range(PAIRS):
                xt = xs.tile([2 * C, HW], BF16, name=f"xt{pr}")
                xts.append(xt)
                ot = osp.tile([2 * C, HW], BF16, name=f"ot{pr}")
                ots.append(ot)

            # Main loop: per pair, 7 matmul chunks (one PSUM bank each) +
            # bias epilogue alternating ACT/DVE, then the store. Stores are
            # queued on SP *after* all loads, so a store waiting on compute
            # never blocks a load. (nreps>1 repeats the full pipeline —
            # loads included — for slope-based timing.)
            # Boundary pairs stream at chunk-pair granularity: pair 0 so
            # its first store (the write stream) starts ~2 chunks after
            # the first bytes land, pair 7 so the tail chain (last load ->
            # last compute -> last store) is ~1 chunk-pair long. Subtile
            # deps let each chunk's matmul run as soon as its columns land.
            QUARTERS = [(0, 2 * CHUNK), (2 * CHUNK, 2 * CHUNK),
                        (4 * CHUNK, 2 * CHUNK), (6 * CHUNK, CHUNK)]
            HALVES = [(0, 4 * CHUNK), (4 * CHUNK, 3 * CHUNK)]
            # pair 0 at halves (quarters would starve the stream head: the
            # 650ns/issue DGE latency exceeds a quarter's transfer time);
            # pair 7 at quarters (tail: issues are long since pipelined).
            split = {0: HALVES, PAIRS - 1: QUARTERS}
            # DMA issue order interleaves store-pr between load-(pr+4) and
            # load-(pr+5): the HW ring processes transfers roughly in issue
            # order, so all-loads-then-all-stores would run the read and
            # write streams back-to-back instead of duplex. Lag 3 keeps a
            # ~3-transfer margin between a store's data being ready and its
            # queue turn, so its wait never blocks the head even when
            # compute lags under machine load (A/B-measured faster than the
            # tighter lag-1 twice, same direction; model zero-gap).
            STORE_LAG = 3

            def emit_load(pr):
                if pr in split:
                    for off, w in split[pr]:
                        sl = bass.ds(off, w)
                        nc.sync.dma_start(xts[pr][:, sl], x_d[pr][:, sl])
                else:
                    nc.sync.dma_start(xts[pr], x_d[pr])

            def emit_store(pr):
                if pr in split:
                    for off, w in split[pr]:
                        sl = bass.ds(off, w)
                        nc.sync.dma_start(out_d[pr][:, sl], ots[pr][:, sl])
                else:
                    nc.sync.dma_start(out_d[pr], ots[pr])

            def emit_compute(pr):
                for c in range(N_CHUNKS):
                    ps = pp.tile([2 * C, CHUNK], F32)
                    sl = bass.ds(c * CHUNK, CHUNK)
                    nc.tensor.matmul(
                        ps,
                        bd[:, pr, :],
                        xts[pr][:, sl],
                        start=True,
                        stop=True,
                    )
                    if c % 2 == 0:
                        nc.scalar.activation(
                            ots[pr][:, sl],
                            ps,
                            mybir.ActivationFunctionType.Identity,
                            bias=bias2[:, pr : pr + 1],
                        )
                    else:
                        nc.vector.tensor_scalar_add(
                            ots[pr][:, sl], ps, bias2[:, pr : pr + 1]
                        )

            for _ in range(nreps):
                # Software-pipelined emission: store-pr must be EMITTED
                # after compute-pr (the tile framework derives waits from
                # program order) while landing in the SP queue between
                # load-(pr+4) and load-(pr+5) for ring interleaving.
                for pr in range(PAIRS):
                    emit_load(pr)
                    if pr >= 1:
                        emit_compute(pr - 1)
                    if pr >= STORE_LAG + 1:
                        emit_store(pr - STORE_LAG - 1)
                emit_compute(PAIRS - 1)
                for pr in range(PAIRS - STORE_LAG - 1, PAIRS):
                    emit_store(pr)

    if legalize:
        _legalize_waits(nc)
    return nc


_NC = None


def _get_program():
    global _NC
    if _NC is None:
        _NC = build_program()
    return _NC


def make_in_maps(x, weights, indices, W_shared, b_shared, W_routed, b_routed):
    x = np.asarray(x, dtype=np.float32)
    weights = np.asarray(weights, dtype=np.float32)
    indices = np.asarray(indices, dtype=np.int32)
    W_shared = np.asarray(W_shared, dtype=np.float32)
    b_shared = np.asarray(b_shared, dtype=np.float32)
    W_routed = np.asarray(W_routed, dtype=np.float32)
    b_routed = np.asarray(b_routed, dtype=np.float32)

    # Host-side routing: combined per-sample weights and biases (tiny).
    Wc = W_shared[None] + weights[:, None, None] * W_routed[indices]  # [B,o,i]
    bc = b_shared[None] + weights[:, None] * b_routed[indices]        # [B,o]

    xb = np.ascontiguousarray(x.reshape(B, C, HW)).astype(NP_BF16)

    in_maps = []
    for i in range(N_CORES):
        lo, hi = i * B_LOC, (i + 1) * B_LOC
        Wl = Wc[lo:hi]  # [16, o, i]
        # Block-diagonal lhsT bank bd[k, pr, m]: sample 2*pr occupies the
        # top-left 64x64 (as [i, o], transposed for lhsT), sample 2*pr+1
        # the bottom-right.
        bd = np.zeros((2 * C, PAIRS, 2 * C), dtype=np.float32)
        bd[:C, :, :C] = Wl[0::2].transpose(2, 0, 1)
        bd[C:, :, C:] = Wl[1::2].transpose(2, 0, 1)
        bl = bc[lo:hi]  # [16, o]
        bias2 = np.concatenate([bl[0::2].T, bl[1::2].T], axis=0)  # [128, 8]
        in_maps.append(
            {
                "x": xb[lo:hi].reshape(PAIRS, 2 * C, HW),
                "bd": bd.astype(NP_BF16),
                "bias2": np.ascontiguousarray(bias2, dtype=np.float32),
            }
        )
    return in_maps


def kernel(x, weights, indices, W_shared, b_shared, W_routed, b_routed):
    out, _ = _run(
        x, weights, indices, W_shared, b_shared, W_routed, b_routed, trace=False
    )
    return out


def kernel_traced(x, weights, indices, W_shared, b_shared, W_routed, b_routed):
    """Like kernel() but returns (out, BassKernelResults) with profiling."""
    return _run(
        x, weights, indices, W_shared, b_shared, W_routed, b_routed, trace=True
    )


def _run(x, weights, indices, W_shared, b_shared, W_routed, b_routed, trace):
    nc = _get_program()
    in_maps = make_in_maps(
        x, weights, indices, W_shared, b_shared, W_routed, b_routed
    )
    res = run_bass_kernel_spmd(nc, in_maps, list(range(N_CORES)), trace=trace)
    out = np.empty((B, C, H, W_), dtype=np.float32)
    for i in range(N_CORES):
        lo, hi = i * B_LOC, (i + 1) * B_LOC
        out[lo:hi] = (
            res.results[i]["out"].astype(np.float32).reshape(B_LOC, C, H, W_)
        )
    return out, res



# revision 5
# speedup vs baseline: 1.2518x; 1.2518x over previous
"""Trainium2 Bass kernel for a decoupled-MoE 1x1-conv container.

Math (per sample b):
    out[b] = (W_shared + weights[b] * W_routed[idx[b]]) @ x[b]
             + (b_shared + weights[b] * b_routed[idx[b]])

Strategy: data-parallel over batch B=128 across 8 NeuronCores (16 samples
per core). The expert gather/combine is a tiny O(B*C^2) host-side
computation over host-visible routing inputs; each core receives its
per-pair block-diagonal combined-weight bank [128, 8, 128] (2 samples per
128x128 lhsT tile) plus a bias bank, both precomputed on host. Activations
stream through the core in bf16 (tolerance is 2e-2; bf16 end-to-end error
is ~4e-3), which halves the HBM traffic of the memory-bound main loop:
~12.9 MB per core at ~360 GB/s. All x tiles, out tiles, and the weight
bank fit in SBUF simultaneously (~13 MB), so the DMA pipeline has no
buffer-recycle stalls: loads are issued up front and the store queue never
blocks a load.
"""

import numpy as np
import ml_dtypes

import concourse.bass as bass
import concourse.mybir as mybir
import concourse.tile as tile
from concourse.bass_utils import run_bass_kernel_spmd

F32 = mybir.dt.float32
BF16 = mybir.dt.bfloat16
F8E3 = mybir.dt.float8e3  # e3m4: 4 mantissa bits, max 15.5
NP_BF16 = ml_dtypes.bfloat16
NP_F8E3 = ml_dtypes.float8_e3m4

N_CORES = 8
B = 128
C = 64  # C_IN == C_OUT == 64
H = 56
W_ = 56
HW = H * W_  # 3136
B_LOC = B // N_CORES  # 16 samples per core
PAIRS = B_LOC // 2  # 8 pairs -> [128, HW] tiles
CHUNK = 448  # 7 chunks of 448 = 3136, one PSUM bank each
N_CHUNKS = HW // CHUNK


def _legalize_waits(nc, dma_limit=1):
    """Walrus on this target allows a single sync-wait slot per engine
    compute instruction (sequencer-only instructions like InstDrain take
    many). Split excess waits onto same-engine NOPs inserted just before
    the offending instruction — semantically identical: the engine queue
    blocks on each wait in turn before executing the instruction."""
    import bass_rust

    counter = [0]
    for fn in nc.m.functions:
        for blk in fn.blocks:
            new_insts = []
            for inst in blk.instructions:
                si = inst.sync_info
                tname = type(inst).__name__
                limit = dma_limit if tname == "InstDMACopy" else 1
                if si is not None and si.on_wait and len(si.on_wait) > limit:
                    waits = list(si.on_wait)
                    keep = waits[-limit:]
                    extra = waits[:-limit]
                    for w in extra:
                        nop = mybir.InstNoOp(
                            name=f"lgl-nop-{counter[0]}", ins=[], outs=[]
                        )
                        counter[0] += 1
                        nop.engine = inst.engine
                        nop.sync_info = bass_rust.SyncInfo(
                            on_wait=[w], on_update=[]
                        )
                        new_insts.append(nop)
                    si.on_wait = keep
                new_insts.append(inst)
            blk.instructions = new_insts
    return nc


def build_program(legalize=True, nreps=1):
    nc = bass.Bass(
        "TRN2", target_bir_lowering=False, debug=False, use_seq_codegen=True
    )

    x_d = nc.dram_tensor("x", [PAIRS, 2 * C, HW], F8E3, kind="ExternalInput")
    bd_d = nc.dram_tensor("bd", [2 * C, PAIRS, 2 * C], BF16, kind="ExternalInput")
    bias_d = nc.dram_tensor("bias2", [2 * C, PAIRS], F32, kind="ExternalInput")
    out_d = nc.dram_tensor("out", [PAIRS, 2 * C, HW], BF16, kind="ExternalOutput")

    with tile.TileContext(nc) as tc:
        with (
            tc.tile_pool(name="keep", bufs=1) as keep,
            tc.tile_pool(name="xs", bufs=1) as xs,
            tc.tile_pool(name="os", bufs=1) as osp,
            tc.tile_pool(name="pp", bufs=8, space="PSUM") as pp,
        ):
            # Weight/bias banks: tiny; issued from the ACT queue so the SP
            # queue's first issue is already x0 (keeps the big-DMA stream
            # head as early as possible).
            bd = keep.tile([2 * C, PAIRS, 2 * C], BF16)
            nc.scalar.dma_start(bd, bd_d.ap())
            bias2 = keep.tile([2 * C, PAIRS], F32)
            nc.scalar.dma_start(bias2, bias_d.ap())

            # PE p-state warmup: the PE ramps (pstate low/mid) for its first
            # ~3us of activity, which would otherwise slow pair-0's chunks
            # and delay the first store (the write stream's start). Burn the
            # idle head window with dummy full-width matmuls. Sized to 7 so
            # the chain ends (~4.1us) just past the EARLIEST plausible HW
            # arrival of x0+bd (~3.4us with duplex DMA): it bridges into the
            # first real matmul when loads are fast, and merely idles out —
            # costing nothing — when they are slow. A longer chain could
            # overshoot a fast gate and delay pair 0 more than the warm
            # chunks save.
            ws = keep.tile([2 * C, CHUNK], BF16)
            nc.gpsimd.memset(ws, 0.0)
            # Warmup shares the main chunk tag ("ps") so its banks return
            # to the 8-deep rotation; the WAW reuse deps are satisfied long
            # before the main loop reaches them.
            for _ in range(7):
                wps = pp.tile([2 * C, CHUNK], F32, name="ps")
                nc.tensor.matmul(
                    wps[:8, :], ws[:, :8], ws, start=True, stop=True
                )

            # All x tiles persistent in SBUF: issue every load up front on
            # the SP queue; nothing ever waits to reuse these buffers.
            xts = []
            ots = []
            for pr in range(PAIRS):
                xt = xs.tile([2 * C, HW], F8E3, name=f"xt{pr}")
                xts.append(xt)
                ot = osp.tile([2 * C, HW], BF16, name=f"ot{pr}")
                ots.append(ot)

            # Main loop: per pair, 7 matmul chunks (one PSUM bank each) +
            # bias epilogue alternating ACT/DVE, then the store. Stores are
            # queued on SP *after* all loads, so a store waiting on compute
            # never blocks a load. (nreps>1 repeats the full pipeline —
            # loads included — for slope-based timing.)
            # Boundary pairs stream at chunk-pair granularity: pair 0 so
            # its first store (the write stream) starts ~2 chunks after
            # the first bytes land, pair 7 so the tail chain (last load ->
            # last compute -> last store) is ~1 chunk-pair long. Subtile
            # deps let each chunk's matmul run as soon as its columns land.
            QUARTERS = [(0, 2 * CHUNK), (2 * CHUNK, 2 * CHUNK),
                        (4 * CHUNK, 2 * CHUNK), (6 * CHUNK, CHUNK)]
            HALVES = [(0, 4 * CHUNK), (4 * CHUNK, 3 * CHUNK)]
            # pair 0 at halves (quarters would starve the stream head: the
            # 650ns/issue DGE latency exceeds a quarter's transfer time);
            # pair 7 at quarters (tail: issues are long since pipelined).
            split = {0: HALVES, PAIRS - 1: QUARTERS}
            # DMA issue order interleaves store-pr between load-(pr+4) and
            # load-(pr+5): the HW ring processes transfers roughly in issue
            # order, so all-loads-then-all-stores would run the read and
            # write streams back-to-back instead of duplex. Lag 3 keeps a
            # ~3-transfer margin between a store's data being ready and its
            # queue turn, so its wait never blocks the head even when
            # compute lags under machine load (A/B-measured faster than the
            # tighter lag-1 twice, same direction; model zero-gap).
            STORE_LAG = 3

            def emit_load(pr):
                if pr in split:
                    for off, w in split[pr]:
                        sl = bass.ds(off, w)
                        nc.sync.dma_start(xts[pr][:, sl], x_d[pr][:, sl])
                else:
                    nc.sync.dma_start(xts[pr], x_d[pr])

            def emit_store(pr):
                if pr in split:
                    for off, w in split[pr]:
                        sl = bass.ds(off, w)
                        nc.sync.dma_start(out_d[pr][:, sl], ots[pr][:, sl])
                else:
                    nc.sync.dma_start(out_d[pr], ots[pr])

            def emit_compute(pr):
                for c in range(N_CHUNKS):
                    ps = pp.tile([2 * C, CHUNK], F32)
                    sl = bass.ds(c * CHUNK, CHUNK)
                    nc.tensor.matmul(
                        ps,
                        bd[:, pr, :],
                        xts[pr][:, sl],
                        start=True,
                        stop=True,
                    )
                    if c % 2 == 0:
                        nc.scalar.activation(
                            ots[pr][:, sl],
                            ps,
                            mybir.ActivationFunctionType.Identity,
                            bias=bias2[:, pr : pr + 1],
                        )
                    else:
                        nc.vector.tensor_scalar_add(
                            ots[pr][:, sl], ps, bias2[:, pr : pr + 1]
                        )

            for _ in range(nreps):
                # Software-pipelined emission: store-pr must be EMITTED
                # after compute-pr (the tile framework derives waits from
                # program order) while landing in the SP queue between
                # load-(pr+4) and load-(pr+5) for ring interleaving.
                for pr in range(PAIRS):
                    emit_load(pr)
                    if pr >= 1:
                        emit_compute(pr - 1)
                    if pr >= STORE_LAG + 1:
                        emit_store(pr - STORE_LAG - 1)
                emit_compute(PAIRS - 1)
                for pr in range(PAIRS - STORE_LAG - 1, PAIRS):
                    emit_store(pr)

    if legalize:
        _legalize_waits(nc)
    return nc


_NC = None


def _get_program():
    global _NC
    if _NC is None:
        _NC = build_program()
    return _NC


def make_in_maps(x, weights, indices, W_shared, b_shared, W_routed, b_routed):
    x = np.asarray(x, dtype=np.float32)
    weights = np.asarray(weights, dtype=np.float32)
    indices = np.asarray(indices, dtype=np.int32)
    W_shared = np.asarray(W_shared, dtype=np.float32)
    b_shared = np.asarray(b_shared, dtype=np.float32)
    W_routed = np.asarray(W_routed, dtype=np.float32)
    b_routed = np.asarray(b_routed, dtype=np.float32)

    # Host-side routing: combined per-sample weights and biases (tiny).
    Wc = W_shared[None] + weights[:, None, None] * W_routed[indices]  # [B,o,i]
    bc = b_shared[None] + weights[:, None] * b_routed[indices]        # [B,o]

    xb = np.ascontiguousarray(x.reshape(B, C, HW)).astype(NP_F8E3)

    in_maps = []
    for i in range(N_CORES):
        lo, hi = i * B_LOC, (i + 1) * B_LOC
        Wl = Wc[lo:hi]  # [16, o, i]
        # Block-diagonal lhsT bank bd[k, pr, m]: sample 2*pr occupies the
        # top-left 64x64 (as [i, o], transposed for lhsT), sample 2*pr+1
        # the bottom-right.
        bd = np.zeros((2 * C, PAIRS, 2 * C), dtype=np.float32)
        bd[:C, :, :C] = Wl[0::2].transpose(2, 0, 1)
        bd[C:, :, C:] = Wl[1::2].transpose(2, 0, 1)
        bl = bc[lo:hi]  # [16, o]
        bias2 = np.concatenate([bl[0::2].T, bl[1::2].T], axis=0)  # [128, 8]
        in_maps.append(
            {
                "x": xb[lo:hi].reshape(PAIRS, 2 * C, HW),
                "bd": bd.astype(NP_BF16),
                "bias2": np.ascontiguousarray(bias2, dtype=np.float32),
            }
        )
    return in_maps


def kernel(x, weights, indices, W_shared, b_shared, W_routed, b_routed):
    out, _ = _run(
        x, weights, indices, W_shared, b_shared, W_routed, b_routed, trace=False
    )
    return out


def kernel_traced(x, weights, indices, W_shared, b_shared, W_routed, b_routed):
    """Like kernel() but returns (out, BassKernelResults) with profiling."""
    return _run(
        x, weights, indices, W_shared, b_shared, W_routed, b_routed, trace=True
    )


def _run(x, weights, indices, W_shared, b_shared, W_routed, b_routed, trace):
    nc = _get_program()
    in_maps = make_in_maps(
        x, weights, indices, W_shared, b_shared, W_routed, b_routed
    )
    res = run_bass_kernel_spmd(nc, in_maps, list(range(N_CORES)), trace=trace)
    out = np.empty((B, C, H, W_), dtype=np.float32)
    for i in range(N_CORES):
        lo, hi = i * B_LOC, (i + 1) * B_LOC
        out[lo:hi] = (
            res.results[i]["out"].astype(np.float32).reshape(B_LOC, C, H, W_)
        )
    return out, res



# revision 18
# speedup vs baseline: 1.3080x; 1.0449x over previous
"""Trainium2 Bass kernel for a decoupled-MoE 1x1-conv container.

Math (per sample b):
    out[b] = (W_shared + weights[b] * W_routed[idx[b]]) @ x[b]
             + (b_shared + weights[b] * b_routed[idx[b]])

Strategy: data-parallel over batch B=128 across 8 NeuronCores (16 samples
per core). The expert gather/combine is a tiny O(B*C^2) host-side
computation over host-visible routing inputs; each core receives a DENSE
per-pair combined-weight bank [128, 8, 64] bf16 plus a bias bank, and
expands the weights on-device (memset + 2 strided copies) into the
block-diagonal lhsT bank [128, 8, 128] (2 samples per 128x128 tile) -
shipping the zeros would double the weight-stream bytes.

Activations stream through the core in fp8 e3m4 (4 mantissa bits): the
matmul consumes the fp8 rhs directly against the bf16 lhsT (the PE upcasts
internally), so quantizing x costs no engine work and halves the dominant
load stream. End-to-end max-rel error vs the f32 reference is 1.61e-2
(tolerance 2e-2); outputs stream back in bf16. (fp8 e4m3 x and int8 /
fp8 outputs all breach the tolerance; int8 x would pass but needs an
on-chip dequant pass that does not fit the ACT/DVE budget.)

The kernel is DMA-bound and the cost structure is serial: makespan =
head (queue prologue + first-issue latency) + total-DMA-bytes/BW + tail
(last-store semaphore + exit barrier). Hence the layout choices:
  - loads are per-pair [128, 3136B] transfers issued back-to-back from
    the SP queue starting with x0 (largest early transfer covers the
    625ns/issue HWDGE pipeline ramp); dense weights second; tiny bias
    late. The DMA ring then never idles: zero gaps start-to-end.
  - all x tiles, the out bank, and the weight banks fit in SBUF
    simultaneously, so no buffer-recycle stalls exist anywhere.
  - stores leave in 5 transfers covering [1,2,2,2,1] pairs of the out
    bank (dram layout [2C, PAIRS, HW]): the first is small so it is
    ready by the time the ring drains the loads; the rest are coarse to
    keep the issue count and the exit-barrier semaphore chain short.
  - the ACT/DVE bias epilogue alternates engines per chunk AND flips the
    4-chunk side per pair, keeping both engines ~equally loaded (~52-54%
    busy) so out data always beats its store's ring turn.
"""

import numpy as np
import ml_dtypes

import concourse.bass as bass
import concourse.mybir as mybir
import concourse.tile as tile
from concourse.bass_utils import run_bass_kernel_spmd

F32 = mybir.dt.float32
BF16 = mybir.dt.bfloat16
F8E3 = mybir.dt.float8e3  # e3m4: 4 mantissa bits, max 15.5
NP_BF16 = ml_dtypes.bfloat16
NP_F8E3 = ml_dtypes.float8_e3m4

N_CORES = 8
B = 128
C = 64  # C_IN == C_OUT == 64
H = 56
W_ = 56
HW = H * W_  # 3136
B_LOC = B // N_CORES  # 16 samples per core
PAIRS = B_LOC // 2  # 8 pairs -> [128, HW] tiles
CHUNK = 448  # 7 chunks of 448 = 3136, one PSUM bank each
N_CHUNKS = HW // CHUNK
# Out leaves in 5 transfers covering [1, 2, 2, 2, 1] pairs: the first store
# is small so it is ready by the time the DMA ring drains the loads; the
# rest are coarse to keep the issue count and exit-barrier chain short.
STORE_GROUPS = [(0, 1), (1, 3), (3, 5), (5, 7), (7, 8)]


def _legalize_waits(nc, dma_limit=1):
    """Walrus on this target allows a single sync-wait slot per engine
    compute instruction (sequencer-only instructions like InstDrain take
    many). Split excess waits onto same-engine NOPs inserted just before
    the offending instruction — semantically identical: the engine queue
    blocks on each wait in turn before executing the instruction."""
    import bass_rust

    counter = [0]
    for fn in nc.m.functions:
        for blk in fn.blocks:
            new_insts = []
            for inst in blk.instructions:
                si = inst.sync_info
                tname = type(inst).__name__
                limit = dma_limit if tname == "InstDMACopy" else 1
                if si is not None and si.on_wait and len(si.on_wait) > limit:
                    waits = list(si.on_wait)
                    keep = waits[-limit:]
                    extra = waits[:-limit]
                    for w in extra:
                        nop = mybir.InstNoOp(
                            name=f"lgl-nop-{counter[0]}", ins=[], outs=[]
                        )
                        counter[0] += 1
                        nop.engine = inst.engine
                        nop.sync_info = bass_rust.SyncInfo(
                            on_wait=[w], on_update=[]
                        )
                        new_insts.append(nop)
                    si.on_wait = keep
                new_insts.append(inst)
            blk.instructions = new_insts
    return nc


def build_program(legalize=True, nreps=1):
    nc = bass.Bass(
        "TRN2", target_bir_lowering=False, debug=False, use_seq_codegen=True
    )

    x_d = nc.dram_tensor("x", [PAIRS, 2 * C, HW], F8E3, kind="ExternalInput")
    # Dense weight bank: [i(2C), pair, o(C)] — rows 0..C-1 hold the even
    # sample's W^T, rows C.. the odd sample's. Expanded on-device into the
    # block-diagonal lhsT bank (ships half the bytes of the block-diag).
    bdd_d = nc.dram_tensor("bdd", [2 * C, PAIRS, C], BF16, kind="ExternalInput")
    bias_d = nc.dram_tensor("bias2", [2 * C, PAIRS], F32, kind="ExternalInput")
    out_d = nc.dram_tensor("out", [2 * C, PAIRS, HW], BF16, kind="ExternalOutput")

    with tile.TileContext(nc) as tc:
        with (
            tc.tile_pool(name="keep", bufs=1) as keep,
            tc.tile_pool(name="xs", bufs=1) as xs,
            tc.tile_pool(name="os", bufs=1) as osp,
            tc.tile_pool(name="pp", bufs=8, space="PSUM") as pp,
        ):
            bd = keep.tile([2 * C, PAIRS, 2 * C], BF16)
            bdd = keep.tile([2 * C, PAIRS, C], BF16)
            bias2 = keep.tile([2 * C, PAIRS], F32)

            # PE p-state warmup: the PE ramps (pstate low/mid) for its first
            # ~3us of activity; burn the DMA head window with dummy
            # full-width matmuls so real chunks run at speed. Costs nothing:
            # the PE would otherwise idle until x0+bd land (~4.2us).
            ws = keep.tile([2 * C, CHUNK], BF16)
            nc.gpsimd.memset(ws, 0.0)
            # Zero the block-diag bank early (Pool is otherwise idle); the
            # off-diagonal quadrants stay zero forever.
            nc.gpsimd.memset(bd, 0.0)
            for _ in range(10):
                wps = pp.tile([2 * C, CHUNK], F32, name="ps")
                nc.tensor.matmul(
                    wps[:8, :], ws[:, :8], ws, start=True, stop=True
                )

            # All x tiles and the out bank persist in SBUF; nothing ever
            # waits to reuse a buffer.
            xts = [xs.tile([2 * C, HW], F8E3, name=f"xt{pr}") for pr in range(PAIRS)]
            ot = osp.tile([2 * C, PAIRS, HW], BF16, name="ot")

            def emit_compute(pr):
                for c in range(N_CHUNKS):
                    ps = pp.tile([2 * C, CHUNK], F32)
                    sl = bass.ds(c * CHUNK, CHUNK)
                    nc.tensor.matmul(
                        ps,
                        bd[:, pr, :],
                        xts[pr][:, sl],
                        start=True,
                        stop=True,
                    )
                    # Bias epilogue alternates ACT/DVE (GPSIMD cannot read
                    # PSUM on this target), flipping which engine gets the
                    # 4th chunk each pair so both stay ~equally loaded and
                    # the out stream never lags a store's DMA-ring turn.
                    if (c + pr) % 2 == 0:
                        nc.scalar.activation(
                            ot[:, pr, sl],
                            ps,
                            mybir.ActivationFunctionType.Identity,
                            bias=bias2[:, pr : pr + 1],
                        )
                    else:
                        nc.vector.tensor_scalar_add(
                            ot[:, pr, sl], ps, bias2[:, pr : pr + 1]
                        )

            for _ in range(nreps):
                # DMA issue order = ring processing order. x0 first (its
                # transfer covers the HWDGE issue ramp), bd second (needed
                # before any matmul), tiny bias after x1 (only needed at
                # pair-0's epilogue), remaining loads back-to-back. The two
                # giant stores are emitted after their pairs' epilogues;
                # their DMA-ring turn comes later than the data is ready,
                # so the ring never idles.
                nc.sync.dma_start(xts[0], x_d[0])
                nc.sync.dma_start(bdd, bdd_d.ap())
                nc.sync.dma_start(xts[1], x_d[1])
                nc.sync.dma_start(bias2, bias_d.ap())
                for pr in range(2, PAIRS):
                    nc.sync.dma_start(xts[pr], x_d[pr])
                # Expand dense bank into the zeroed block-diagonal lhsT bank:
                # even samples into the top-left C x C of each pair block,
                # odd into the bottom-right.
                nc.vector.tensor_scalar_add(bd[:C, :, :C], bdd[:C], 0.0)
                nc.scalar.add(bd[C:, :, C:], bdd[C:], 0.0)
                for lo, hi in STORE_GROUPS:
                    for pr in range(lo, hi):
                        emit_compute(pr)
                    sl = bass.ds(lo, hi - lo)
                    nc.sync.dma_start(out_d[:, sl, :], ot[:, sl, :])

    if legalize:
        _legalize_waits(nc)
    return nc


_NC = None


def _get_program():
    global _NC
    if _NC is None:
        _NC = build_program()
    return _NC


def make_in_maps(x, weights, indices, W_shared, b_shared, W_routed, b_routed):
    x = np.asarray(x, dtype=np.float32)
    weights = np.asarray(weights, dtype=np.float32)
    indices = np.asarray(indices, dtype=np.int32)
    W_shared = np.asarray(W_shared, dtype=np.float32)
    b_shared = np.asarray(b_shared, dtype=np.float32)
    W_routed = np.asarray(W_routed, dtype=np.float32)
    b_routed = np.asarray(b_routed, dtype=np.float32)

    # Host-side routing: combined per-sample weights and biases (tiny).
    Wc = W_shared[None] + weights[:, None, None] * W_routed[indices]  # [B,o,i]
    bc = b_shared[None] + weights[:, None] * b_routed[indices]        # [B,o]

    xb = np.ascontiguousarray(x.reshape(B, C, HW)).astype(NP_F8E3)

    in_maps = []
    for i in range(N_CORES):
        lo, hi = i * B_LOC, (i + 1) * B_LOC
        Wl = Wc[lo:hi]  # [16, o, i]
        # Dense lhsT bank bdd[k, pr, o]: rows 0..C-1 = even sample's W^T
        # (indexed by input channel k), rows C.. = odd sample's.
        bdd = np.empty((2 * C, PAIRS, C), dtype=np.float32)
        bdd[:C] = Wl[0::2].transpose(2, 0, 1)
        bdd[C:] = Wl[1::2].transpose(2, 0, 1)
        bl = bc[lo:hi]  # [16, o]
        bias2 = np.concatenate([bl[0::2].T, bl[1::2].T], axis=0)  # [128, 8]
        in_maps.append(
            {
                "x": xb[lo:hi].reshape(PAIRS, 2 * C, HW),
                "bdd": bdd.astype(NP_BF16),
                "bias2": np.ascontiguousarray(bias2, dtype=np.float32),
            }
        )
    return in_maps


def kernel(x, weights, indices, W_shared, b_shared, W_routed, b_routed):
    out, _ = _run(
        x, weights, indices, W_shared, b_shared, W_routed, b_routed, trace=False
    )
    return out


def kernel_traced(x, weights, indices, W_shared, b_shared, W_routed, b_routed):
    """Like kernel() but returns (out, BassKernelResults) with profiling."""
    return _run(
        x, weights, indices, W_shared, b_shared, W_routed, b_routed, trace=True
    )


def _run(x, weights, indices, W_shared, b_shared, W_routed, b_routed, trace):
    nc = _get_program()
    in_maps = make_in_maps(
        x, weights, indices, W_shared, b_shared, W_routed, b_routed
    )
    res = run_bass_kernel_spmd(nc, in_maps, list(range(N_CORES)), trace=trace)
    out = np.empty((B, C, H, W_), dtype=np.float32)
    for i in range(N_CORES):
        lo, hi = i * B_LOC, (i + 1) * B_LOC
        o = res.results[i]["out"].astype(np.float32)  # [2C, PAIRS, HW]
        smp = out[lo:hi].reshape(B_LOC, C, HW)
        smp[0::2] = o[:C].transpose(1, 0, 2)
        smp[1::2] = o[C:].transpose(1, 0, 2)
    return out, res
